# revision 1
# baseline (speedup 1.0000x reference)
"""Self-contained GCN encoder kernel for 8 TRN2 NeuronCores (Bass/Tile).

kernel(**inputs) takes the FULL unsharded inputs (as from setup_inputs())
and returns the FULL [50000, 64] float32 output.

Strategy: dst-node tiles of 128 are LPT-balanced across 8 cores (quantile-
matched slot order keeps the SPMD instruction stream identical); the NEFF is
specialized per run to the edge structure. Per core: embedding gathers +
feat^T assembly -> matmul -> dinv-scaled bf16 h1 table (emb_b handled via a
folded weight W1B = emb_b @ W1[64:128] and onehot matmuls); two split
AllGathers (slot ranges A/B, each table <= 32767 rows for int16 dma_gather
indices) exchange tables between cores; GCN aggregation gathers edge rows
with dma_gather, builds 0/1 selection matrices on-device (is_equal vs iota),
and segment-reduces on the TensorEngine into PSUM. The symmetric norm is
folded into the tables (src side) and the epilogue scale (dst side);
self-loops are a per-tile row add. B-table gather ops trail A-ops by 3
groups (phase-split PSUM accumulation with SBUF spill) so collectives
overlap gather work.
"""
import numpy as np
from concourse import bass, bacc, mybir, tile
from concourse.bass_utils import run_bass_kernel_spmd
from concourse.masks import make_identity

P = 128
CORES = 8
N = 50000
NTILES = 392
NPAD = NTILES * P     # 50176
TPC = NTILES // CORES  # 49
NLOC = TPC * P        # 6272
KSPLIT = 25           # slots 0..24 -> table A, 25..48 -> table B
NA = KSPLIT * P       # 3200 rows per core in A
NB = NLOC - NA        # 3072
TABA = NA * CORES     # 25600
TABB = NB * CORES     # 24576
C1 = 128
C2 = 64
IN_CH = 136
PAD_DSTL = 30000.0
G_MERGE = 4
DELAY_B = 3           # groups of A-ops run before each B-op


def wrap_idx(arr):
    return arr.reshape(-1, 16).T


def rup(x, m):
    return int((x + m - 1) // m * m)


def prep(x, edge_index, emb_a, emb_b, W1, b1, W2, b2):
    src, dst = np.asarray(edge_index[0]), np.asarray(edge_index[1])
    deg = np.bincount(dst, minlength=N).astype(np.float32) + 1.0
    dinv = (1.0 / np.sqrt(deg)).astype(np.float32)

    # ---- tile -> core assignment (LPT on edge counts) ----
    t_of_e = dst // P
    tile_cnt = np.bincount(t_of_e, minlength=NTILES)
    order = np.argsort(-tile_cnt, kind="stable")
    core_loads = np.zeros(CORES, dtype=np.int64)
    core_tiles = [[] for _ in range(CORES)]
    for t in order:
        c = int(np.argmin(core_loads))
        core_tiles[c].append(int(t))
        core_loads[c] += tile_cnt[t]
    c_of_t = np.zeros(NTILES, dtype=np.int64)
    k_of_t = np.zeros(NTILES, dtype=np.int64)
    for c in range(CORES):
        for k, t in enumerate(core_tiles[c]):
            c_of_t[t] = c
            k_of_t[t] = k

    # table coordinates: src node -> (which table, row)
    node_ids = np.arange(NPAD)
    nc_core = c_of_t[node_ids // P]
    nc_slot = k_of_t[node_ids // P]
    in_b = nc_slot >= KSPLIT
    trow = np.where(in_b,
                    nc_core * NB + (nc_slot - KSPLIT) * P + node_ids % P,
                    nc_core * NA + nc_slot * P + node_ids % P)

    # ---- sort edges by (core, slot, table) ----
    e_tab = in_b[src].astype(np.int64)
    e_row = trow[src]
    e_k = k_of_t[t_of_e]
    key = (c_of_t[t_of_e] * TPC + e_k) * 2 + e_tab
    sort = np.argsort(key, kind="stable")
    row_s = e_row[sort]
    dstl_s = (dst % P).astype(np.float32)[sort]
    bounds = np.searchsorted(key[sort], np.arange(CORES * TPC * 2 + 1))

    # ---- op schedule: per (group of G_MERGE slots, table): contiguous pack ----
    raw_ops = []   # (h, k0, k1, num_idxs, idxcol_off, [(j, k, paircol)...])
    idxcol_off = 0
    paircol = 0
    pairs_of_tile = np.zeros(TPC, dtype=np.int64)
    for k0 in range(0, TPC, G_MERGE):
        k1 = min(k0 + G_MERGE, TPC)
        for h in (0, 1):
            m_op = [sum(int(bounds[(c * TPC + k) * 2 + h + 1] -
                            bounds[(c * TPC + k) * 2 + h])
                        for k in range(k0, k1)) for c in range(CORES)]
            n = rup(max(m_op), P) // P
            if n == 0:
                continue
            pairset = set()
            for c in range(CORES):
                off = 0
                for k in range(k0, k1):
                    g = (c * TPC + k) * 2 + h
                    m = int(bounds[g + 1] - bounds[g])
                    if m == 0:
                        continue
                    for j in range(off // P, (off + m - 1) // P + 1):
                        pairset.add((j, k))
                    off += m
            pairlist = []
            for (j, k) in sorted(pairset):
                pairlist.append((j, k, paircol))
                pairs_of_tile[k] += 1
                paircol += 1
            raw_ops.append((h, k0, k1, n * P, idxcol_off, pairlist))
            idxcol_off += n * P // 16
    GCOLS = idxcol_off
    NPAIRS = paircol
    MAXCH = max(op[3] // P for op in raw_ops)
    n_rows_tot = sum(op[3] for op in raw_ops)
    assert all(pairs_of_tile > 0)

    # delayed-B emission order: A-ops stream, B-ops trail by DELAY_B groups
    a_ops = [op for op in raw_ops if op[0] == 0]
    b_ops = [op for op in raw_ops if op[0] == 1]
    ops_seq = []
    bi = 0
    for gi, aop in enumerate(a_ops):
        ops_seq.append(aop)
        if gi >= DELAY_B and bi < len(b_ops):
            ops_seq.append(b_ops[bi])
            bi += 1
    ops_seq.extend(b_ops[bi:])

    # ---- per-core arrays ----
    in_maps = []
    iota = np.tile(np.arange(P, dtype=np.float32)[None, :], (P, 1))
    for c in range(CORES):
        gidx16 = np.zeros((16, GCOLS), dtype=np.int16)
        dstlm = np.full((P, NPAIRS), PAD_DSTL, dtype=np.float32)
        for (h, k0, k1, num_idxs, coff, pairlist) in raw_ops:
            idx = np.zeros(num_idxs, dtype=np.int16)
            tilearr = np.full(num_idxs, -1, dtype=np.int64)
            dl = np.full(num_idxs, PAD_DSTL, dtype=np.float32)
            off = 0
            for k in range(k0, k1):
                g = (c * TPC + k) * 2 + h
                lo, hi = bounds[g], bounds[g + 1]
                m = int(hi - lo)
                if m == 0:
                    continue
                idx[off:off + m] = row_s[lo:hi].astype(np.int16)
                tilearr[off:off + m] = k
                dl[off:off + m] = dstl_s[lo:hi]
                off += m
            gidx16[:, coff:coff + num_idxs // 16] = wrap_idx(idx)
            for (j, k, pc_) in pairlist:
                seg_t = tilearr[j * P:(j + 1) * P]
                seg_d = dl[j * P:(j + 1) * P]
                dstlm[:, pc_] = np.where(seg_t == k, seg_d, PAD_DSTL)
        gidx = np.tile(gidx16, (8, 1))

        nodes = np.concatenate(
            [t * P + np.arange(P) for t in core_tiles[c]])
        valid = nodes < N
        nodes_c = np.where(valid, nodes, 0)
        x_own = np.where(valid[:, None], np.asarray(x)[nodes_c], 0.0).astype(np.float32)
        x_ownT = np.ascontiguousarray(x_own.T).astype(np.float32)
        dinv_own = np.where(valid, dinv[nodes_c], 1.0).astype(np.float32)
        dinvc = dinv_own.reshape(TPC, P).T.copy()

        xa = x_own[:, 0].astype(np.int64)
        eia = np.tile(wrap_idx(xa.astype(np.int16)), (8, 1))
        xbcol = x_own[:, 1].astype(np.float32).reshape(TPC, P).T.copy()

        import ml_dtypes
        in_maps.append({
            "x_ownT": x_ownT[2:10].astype(ml_dtypes.bfloat16),
            "eia": eia.copy(),
            "xbcol": xbcol,
            "emb_a": np.asarray(emb_a, dtype=np.float32),
            "emb_bt": np.asarray(emb_b).T.copy().astype(ml_dtypes.bfloat16),
            "W1b": np.asarray(W1).astype(ml_dtypes.bfloat16),
            "W2": np.asarray(W2, dtype=np.float32),
            "b1f": np.tile(np.asarray(b1, dtype=np.float32)[None, :], (P, 1)),
            "b2f": np.tile(np.asarray(b2, dtype=np.float32)[None, :], (P, 1)),
            "dinvc": dinvc,
            "iota": iota,
            "gidx": gidx,
            "dstlm": dstlm,
        })

    meta = {"raw_ops": raw_ops, "ops_seq": ops_seq, "GCOLS": GCOLS,
            "NPAIRS": NPAIRS, "MAXCH": MAXCH,
            "core_tiles": core_tiles, "n_rows_tot": n_rows_tot}
    return in_maps, meta


def build(meta):
    ops_seq = meta["ops_seq"]
    GCOLS = meta["GCOLS"]
    NPAIRS = meta["NPAIRS"]
    MAXCH = meta["MAXCH"]
    f32 = mybir.dt.float32
    bf16 = mybir.dt.bfloat16
    i16 = mybir.dt.int16
    GS = G_MERGE  # slots per pacc bank tile

    nc = bacc.Bacc("TRN2", target_bir_lowering=False, debug=False,
                   num_devices=CORES)
    x_ownT = nc.dram_tensor("x_ownT", [8, NLOC], bf16, kind="ExternalInput")
    eia = nc.dram_tensor("eia", [P, NLOC // 16], i16, kind="ExternalInput")
    xbcol = nc.dram_tensor("xbcol", [P, TPC], f32, kind="ExternalInput")
    emb_a = nc.dram_tensor("emb_a", [1000, 64], f32, kind="ExternalInput")
    emb_bt = nc.dram_tensor("emb_bt", [64, 50], bf16, kind="ExternalInput")
    W1b = nc.dram_tensor("W1b", [IN_CH, C1], bf16, kind="ExternalInput")
    W2 = nc.dram_tensor("W2", [C1, C2], f32, kind="ExternalInput")
    b1f = nc.dram_tensor("b1f", [P, C1], f32, kind="ExternalInput")
    b2f = nc.dram_tensor("b2f", [P, C2], f32, kind="ExternalInput")
    dinvc = nc.dram_tensor("dinvc", [P, TPC], f32, kind="ExternalInput")
    iota = nc.dram_tensor("iota", [P, P], f32, kind="ExternalInput")
    gidx = nc.dram_tensor("gidx", [P, GCOLS], i16, kind="ExternalInput")
    dstlm = nc.dram_tensor("dstlm", [P, NPAIRS], f32, kind="ExternalInput")
    y = nc.dram_tensor("y", [NLOC, C2], f32, kind="ExternalOutput")

    with tile.TileContext(nc) as tc:
        with tc.tile_pool(name="const", bufs=1) as cpool, \
             tc.tile_pool(name="meta", bufs=1) as mpool, \
             tc.tile_pool(name="emb", bufs=1) as epool, \
             tc.tile_pool(name="feat", bufs=3) as fpool, \
             tc.tile_pool(name="he1", bufs=3) as he1pool, \
             tc.tile_pool(name="he2", bufs=3) as he2pool, \
             tc.tile_pool(name="sel", bufs=4) as spool, \
             tc.tile_pool(name="epi", bufs=3) as tpool, \
             tc.tile_pool(name="part", bufs=30) as partp, \
             tc.tile_pool(name="ptr", bufs=2, space="PSUM") as ptrp, \
             tc.tile_pool(name="pmm", bufs=1, space="PSUM") as pmmp, \
             tc.tile_pool(name="pacc", bufs=5, space="PSUM") as paccp, \
             tc.tile_pool(name="dram", bufs=1, space="DRAM") as dram:

            # ---------- constants ----------
            ident = cpool.tile([P, P], f32, tag="ident")
            make_identity(nc, ident[:])
            identb = cpool.tile([P, P], bf16, tag="identb")
            nc.vector.tensor_copy(out=identb[:], in_=ident[:])
            iota_t = cpool.tile([P, P], f32, tag="iota")
            nc.sync.dma_start(out=iota_t[:], in_=iota[:])
            W1lo = cpool.tile([P, C1], bf16, tag="w1lo")
            nc.sync.dma_start(out=W1lo[:], in_=W1b[0:P, :])
            W1hi = cpool.tile([IN_CH - P, C1], bf16, tag="w1hi")
            nc.sync.dma_start(out=W1hi[:], in_=W1b[P:IN_CH, :])
            W2t = cpool.tile([C1, C2], f32, tag="w2")
            nc.sync.dma_start(out=W2t[:], in_=W2[:])
            b1t = cpool.tile([P, C1], f32, tag="b1")
            nc.sync.dma_start(out=b1t[:], in_=b1f[:])
            b2t = cpool.tile([P, C2], f32, tag="b2")
            nc.sync.dma_start(out=b2t[:], in_=b2f[:])
            dinv_t = cpool.tile([P, TPC], f32, tag="dinv")
            nc.sync.dma_start(out=dinv_t[:], in_=dinvc[:])
            xb_t = cpool.tile([P, TPC], f32, tag="xb")
            nc.sync.dma_start(out=xb_t[:], in_=xbcol[:])
            embBT = cpool.tile([64, 50], bf16, tag="embBT")
            nc.sync.dma_start(out=embBT[:], in_=emb_bt[:])
            eia_t = mpool.tile([P, NLOC // 16], i16, tag="eia")
            nc.sync.dma_start(out=eia_t[:], in_=eia[:])

            agA1 = dram.tile([NA, C1], bf16, tag="agA1")
            agB1 = dram.tile([NB, C1], bf16, tag="agB1")
            tabA1 = dram.tile([TABA, C1], bf16, tag="tabA1")
            tabB1 = dram.tile([TABB, C1], bf16, tag="tabB1")
            agA2 = dram.tile([NA, C2], f32, tag="agA2")
            agB2 = dram.tile([NB, C2], f32, tag="agB2")
            tabA2 = dram.tile([TABA, C2], f32, tag="tabA2")
            tabB2 = dram.tile([TABB, C2], f32, tag="tabB2")

            def slot_dst1(k):
                return (agA1, k * P) if k < KSPLIT else (agB1, (k - KSPLIT) * P)

            def slot_dst2(k):
                return (agA2, k * P) if k < KSPLIT else (agB2, (k - KSPLIT) * P)

            # ---------- stage 1 ----------
            ga = epool.tile([P, TPC * 64], f32, tag="ga")
            gab = epool.tile([P, TPC * 64], bf16, tag="gab")
            spe = 7
            for e in range((TPC + spe - 1) // spe):
                k0e, k1e = e * spe, min((e + 1) * spe, TPC)
                nn = (k1e - k0e) * P
                nc.gpsimd.dma_gather(
                    out_ap=ga[:, k0e * 64:k1e * 64].rearrange(
                        "p (n c) -> p n c", c=64),
                    in_ap=emb_a[:],
                    idxs_ap=eia_t[:, k0e * P // 16:k1e * P // 16],
                    num_idxs=nn, num_idxs_reg=nn, elem_size=64,
                    single_packet=(nn <= 1024))
                nc.vector.tensor_copy(out=gab[:, k0e * 64:k1e * 64],
                                      in_=ga[:, k0e * 64:k1e * 64])
            W1mid = cpool.tile([64, C1], bf16, tag="w1mid")
            nc.sync.dma_start(out=W1mid[:], in_=W1b[64:128, :])
            pWB = ptrp.tile([P, P], f32, space="PSUM", tag="ptr")
            nc.tensor.matmul(out=pWB[0:50, :], lhsT=embBT[:],
                             rhs=W1mid[:], start=True, stop=True)
            W1Bp = cpool.tile([50, C1], bf16, tag="w1bp")
            nc.vector.tensor_copy(out=W1Bp[:], in_=pWB[0:50, :])
            gidx_t = mpool.tile([P, GCOLS], i16, tag="gidx")
            nc.sync.dma_start(out=gidx_t[:], in_=gidx[:])
            dstl_t = mpool.tile([P, NPAIRS], f32, tag="dstl")
            nc.sync.dma_start(out=dstl_t[:], in_=dstlm[:])

            for k in range(TPC):
                ptrA = ptrp.tile([P, P], bf16, space="PSUM", tag="ptr")
                nc.tensor.transpose(out=ptrA[0:64, :], in_=gab[:, k * 64:(k + 1) * 64],
                                    identity=identb[:])
                gaT = fpool.tile([64, P], bf16, tag="gaT")
                nc.vector.tensor_copy(out=gaT[:], in_=ptrA[0:64, :])
                oneB = fpool.tile([P, 64], bf16, tag="oneB")
                nc.vector.tensor_tensor(
                    out=oneB[:, 0:50], in0=xb_t[:, k:k + 1].to_broadcast([P, 50]),
                    in1=iota_t[:, 0:50], op=mybir.AluOpType.is_equal)
                pB = ptrp.tile([P, P], bf16, space="PSUM", tag="ptr")
                nc.tensor.transpose(out=pB[0:50, :], in_=oneB[:, 0:50],
                                    identity=identb[:])
                oneBT = fpool.tile([50, P], bf16, tag="oneBT")
                nc.vector.tensor_copy(out=oneBT[:], in_=pB[0:50, :])
                fThi = fpool.tile([8, P], bf16, tag="fthi")
                nc.sync.dma_start(out=fThi[:], in_=x_ownT[:, k * P:(k + 1) * P])
                ph1 = pmmp.tile([P, C1], f32, space="PSUM", tag="pmm")
                nc.tensor.matmul(out=ph1[:], lhsT=gaT[:], rhs=W1lo[0:64, :],
                                 start=True, stop=False)
                nc.tensor.matmul(out=ph1[:], lhsT=oneBT[:], rhs=W1Bp[:],
                                 start=False, stop=False)
                nc.tensor.matmul(out=ph1[:], lhsT=fThi[:], rhs=W1hi[:],
                                 start=False, stop=True)
                h1s = tpool.tile([P, C1], bf16, tag="h1s")
                nc.scalar.activation(out=h1s[:], in_=ph1[:],
                                     func=mybir.ActivationFunctionType.Copy,
                                     scale=dinv_t[:, k:k + 1])
                dstt, off = slot_dst1(k)
                nc.sync.dma_start(out=dstt[off:off + P, :], in_=h1s[:])
                if k == KSPLIT - 1:
                    nc.gpsimd.collective_compute(
                        "AllGather", mybir.AluOpType.bypass,
                        replica_groups=[list(range(CORES))],
                        ins=[agA1.opt()], outs=[tabA1.opt()])
            nc.gpsimd.collective_compute(
                "AllGather", mybir.AluOpType.bypass,
                replica_groups=[list(range(CORES))],
                ins=[agB1.opt()], outs=[tabB1.opt()])

            # ---------- conv passes ----------
            def conv(tabA, tabB, TA, TB, slot_dst, C, hepool, hetag, hedt, Sdt,
                     btile, last, agg_next=None, fire_b=None):
                npairs_of = {0: {}, 1: {}}
                for op in ops_seq:
                    for (j, k, pc_) in op[5]:
                        d = npairs_of[op[0]]
                        d[k] = d.get(k, 0) + 1
                done_of = {0: {k: 0 for k in npairs_of[0]},
                           1: {k: 0 for k in npairs_of[1]}}
                bank_of = {}      # (k, phase) -> psum tile
                partial_of = {}   # k -> sbuf partial from phase A

                def epilogue(k, pacc_ap):
                    srct, soff = slot_dst(k)
                    self_sb = tpool.tile([P, C], hedt, tag=f"self{C}",
                                         name=f"self_{C}_{k}")
                    nc.sync.dma_start(out=self_sb[:], in_=srct[soff:soff + P, :])
                    t1 = tpool.tile([P, C], f32, tag=f"t1{C}", name=f"t1_{C}_{k}")
                    nc.vector.tensor_add(out=t1[:], in0=pacc_ap, in1=self_sb[:])
                    if k in partial_of:
                        t1b = tpool.tile([P, C], f32, tag=f"t1b{C}",
                                         name=f"t1b_{C}_{k}")
                        nc.vector.tensor_add(out=t1b[:], in0=t1[:],
                                             in1=partial_of.pop(k)[:])
                        t1 = t1b
                    t2 = tpool.tile([P, C], f32, tag=f"t2{C}", name=f"t2_{C}_{k}")
                    nc.scalar.activation(out=t2[:], in_=t1[:],
                                         func=mybir.ActivationFunctionType.Copy,
                                         scale=dinv_t[:, k:k + 1])
                    t3 = tpool.tile([P, C], f32, tag=f"t3{C}", name=f"t3_{C}_{k}")
                    nc.vector.tensor_add(out=t3[:], in0=t2[:], in1=btile[:])
                    hrelu = tpool.tile([P, C], f32, tag=f"hr{C}", name=f"hr_{C}_{k}")
                    nc.vector.tensor_scalar_max(out=hrelu[:], in0=t3[:],
                                                scalar1=0.0)
                    if not last:
                        ptr2 = ptrp.tile([P, P], f32, space="PSUM", tag="ptr",
                                         name=f"ptr2_{k}")
                        nc.tensor.transpose(out=ptr2[:], in_=hrelu[:],
                                            identity=ident[:])
                        hT = fpool.tile([P, P], f32, tag="hT", name=f"hT_{k}")
                        nc.vector.tensor_copy(out=hT[:], in_=ptr2[:])
                        ph2 = pmmp.tile([P, C2], f32, space="PSUM", tag="pmm",
                                        name=f"ph2_{k}")
                        nc.tensor.matmul(out=ph2[:], lhsT=hT[:], rhs=W2t[:],
                                         start=True, stop=True)
                        h2s = tpool.tile([P, C2], f32, tag="h2s", name=f"h2s_{k}")
                        nc.scalar.activation(
                            out=h2s[:], in_=ph2[:],
                            func=mybir.ActivationFunctionType.Copy,
                            scale=dinv_t[:, k:k + 1])
                        d2, o2 = slot_dst2(k)
                        nc.sync.dma_start(out=d2[o2:o2 + P, :], in_=h2s[:])
                        if agg_next is not None:
                            agg_next(k)
                    else:
                        nc.sync.dma_start(out=y[k * P:(k + 1) * P, :],
                                          in_=hrelu[:])

                for opi, (h, k0, k1, num_idxs, coff, pairlist) in enumerate(ops_seq):
                    he = hepool.tile([P, MAXCH * C], hedt, tag=hetag,
                                     name=f"he_{C}_{h}_{k0}")
                    tab = tabB if h else tabA
                    nch = num_idxs // P
                    nc.gpsimd.dma_gather(
                        out_ap=he[:, 0:nch * C].rearrange(
                            "p (n c) -> p n c", c=C),
                        in_ap=tab[:],
                        idxs_ap=gidx_t[:, coff:coff + num_idxs // 16],
                        num_idxs=num_idxs, num_idxs_reg=num_idxs, elem_size=C,
                        single_packet=(num_idxs <= 1024))
                    for (j, k, pc_) in pairlist:
                        ph = h
                        if (k, ph) not in bank_of:
                            bank_of[(k, ph)] = paccp.tile(
                                [P, C], f32, space="PSUM",
                                tag="pacc", name=f"pacc_{C}_{ph}_{k}")
                        pacc_ap = bank_of[(k, ph)][:]
                        S = spool.tile([P, P], Sdt, tag=f"S{C}",
                                       name=f"S_{C}_{pc_}")
                        nc.vector.tensor_tensor(
                            out=S[:],
                            in0=dstl_t[:, pc_:pc_ + 1].to_broadcast([P, P]),
                            in1=iota_t[:],
                            op=mybir.AluOpType.is_equal)
                        nc.tensor.matmul(out=pacc_ap, lhsT=S[:],
                                         rhs=he[:, j * C:(j + 1) * C],
                                         start=(done_of[ph][k] == 0),
                                         stop=(done_of[ph][k] == npairs_of[ph][k] - 1))
                        done_of[ph][k] += 1
                        if done_of[ph][k] == npairs_of[ph][k]:
                            bank_of.pop((k, ph))
                            if ph == 0 and npairs_of[1].get(k, 0) > 0:
                                part = partp.tile([P, C], f32, tag=f"part{C}",
                                                  name=f"part_{C}_{k}")
                                nc.vector.tensor_copy(out=part[:], in_=pacc_ap)
                                partial_of[k] = part
                            else:
                                epilogue(k, pacc_ap)

                for hh in (0, 1):
                    assert all(done_of[hh][k] == npairs_of[hh][k]
                               for k in npairs_of[hh])
                assert not partial_of

            # between-conv collectives, fired as soon as enough slots finished
            def agg_next(k):
                if k == KSPLIT - 1:
                    nc.gpsimd.collective_compute(
                        "AllGather", mybir.AluOpType.bypass,
                        replica_groups=[list(range(CORES))],
                        ins=[agA2.opt()], outs=[tabA2.opt()])
                if k == TPC - 1:
                    nc.gpsimd.collective_compute(
                        "AllGather", mybir.AluOpType.bypass,
                        replica_groups=[list(range(CORES))],
                        ins=[agB2.opt()], outs=[tabB2.opt()])

            conv(tabA1, tabB1, TABA, TABB, slot_dst1, C1, he1pool, "he1",
                 bf16, bf16, b1t, last=False, agg_next=agg_next)
            conv(tabA2, tabB2, TABA, TABB, slot_dst2, C2, he2pool, "he2",
                 f32, f32, b2t, last=True)

    nc.compile()
    return nc


_cache = {}


def kernel(x, edge_index, emb_a, emb_b, W1, b1, W2, b2):
    in_maps, meta = prep(x, edge_index, emb_a, emb_b, W1, b1, W2, b2)
    key = (meta["GCOLS"], meta["NPAIRS"],
           tuple((op[0], op[1], op[2], op[3], op[4], tuple(op[5]))
                 for op in meta["ops_seq"]))
    if key not in _cache:
        _cache[key] = build(meta)
    nc = _cache[key]
    res = run_bass_kernel_spmd(nc, in_maps, core_ids=list(range(CORES)))
    out = np.zeros((N, C2), dtype=np.float32)
    for c in range(CORES):
        yc = res.results[c]["y"]
        nodes = np.concatenate(
            [t * P + np.arange(P) for t in meta["core_tiles"][c]])
        valid = nodes < N
        out[nodes[valid]] = yc[valid]
    return out



# revision 3
# speedup vs baseline: 1.2387x; 1.2387x over previous
"""Self-contained GCN encoder kernel for 8 TRN2 NeuronCores (Bass/Tile).

kernel(**inputs) takes the FULL unsharded inputs (as from setup_inputs())
and returns the FULL [50000, 64] float32 output.

Strategy: stage 1 (embedding + W1) is REPLICATED on every core via a fused
host-precomputed lookup table emb_ab = emb_a@W1[:64] (+) emb_b@W1[64:128]
(one dma_gather per 8-tile group, accumulated into PSUM with an
identity-matmul, plus the numeric-feature matmul), writing the full
dinv-scaled h1 table to local DRAM -- no first AllGather, so the slow
startup CC barrier overlaps compute.  Conv aggregations shard dst-node
tiles across cores (LPT-balanced, quantile-matched slot order keeps the
SPMD stream identical); per-edge rows are fetched with dma_gather striped
over 4 SWDGE queues (4x descriptor-generation throughput) using a
mid-table base pointer and signed int16 indices (no A/B table split).
Seg-reduction is one-hot (is_equal) S-matrices x gathered rows on the
TensorEngine into PSUM; the symmetric norm is folded into table rows (src)
and the epilogue scale (dst); conv1 self-loop rows ride along as an extra
gather chunk, conv2 self-loop terms are stashed in SBUF from the conv1
epilogue.  One AllGather (h2 table) runs between the convs.
"""
import numpy as np
from concourse import bacc, mybir, tile
from concourse.bass_utils import run_bass_kernel_spmd
from concourse.masks import make_identity

P = 128
CORES = 8
N = 50000
NTILES = 392
NPAD = NTILES * P      # 50176
TPC = NTILES // CORES  # 49
NLOC = TPC * P         # 6272
C1 = 128
C2 = 64
EMB_MID = 25000
T1_MID = NPAD // 2     # 25088
T2_MID = NPAD // 2
PAD_DSTL = 30000.0
GS = 8                 # tiles per stage-1 gather op
NQ = 4                 # SWDGE queues

f32 = mybir.dt.float32
bf16 = mybir.dt.bfloat16
i16 = mybir.dt.int16


def wrap_idx(arr):
    return arr.reshape(-1, 16).T


def rup(x, m):
    return int((x + m - 1) // m * m)


def prep(x, edge_index, emb_a, emb_b, W1, b1, W2, b2):
    import ml_dtypes
    x = np.asarray(x)
    src, dst = np.asarray(edge_index[0]).astype(np.int64), \
        np.asarray(edge_index[1]).astype(np.int64)
    deg = np.bincount(dst, minlength=N).astype(np.float32) + 1.0
    dinv = np.ones(NPAD, dtype=np.float32)
    dinv[:N] = 1.0 / np.sqrt(deg)

    # ---- tile -> core assignment (LPT on edge counts) ----
    t_of_e = dst // P
    tile_cnt = np.bincount(t_of_e, minlength=NTILES)
    order = np.argsort(-tile_cnt, kind="stable")
    core_loads = np.zeros(CORES, dtype=np.int64)
    core_tiles = [[] for _ in range(CORES)]
    for t in order:
        c = int(np.argmin(core_loads))
        core_tiles[c].append(int(t))
        core_loads[c] += tile_cnt[t]
    c_of_t = np.zeros(NTILES, dtype=np.int64)
    k_of_t = np.zeros(NTILES, dtype=np.int64)
    for c in range(CORES):
        for k, t in enumerate(core_tiles[c]):
            c_of_t[t] = c
            k_of_t[t] = k

    node_ids = np.arange(NPAD)
    trow2 = c_of_t[node_ids // P] * NLOC + k_of_t[node_ids // P] * P \
        + node_ids % P

    # ---- sort edges by (core, slot) ----
    key = c_of_t[t_of_e] * TPC + k_of_t[t_of_e]
    sort = np.argsort(key, kind="stable")
    src_s = src[sort]
    trow2_s = trow2[src_s]
    dstl_s = (dst % P).astype(np.float32)[sort]
    bounds = np.searchsorted(key[sort], np.arange(CORES * TPC + 1))

    # ---- op schedule: one op per slot k; nch = max over cores ----
    nch_of_k = []
    for k in range(TPC):
        m = max(int(bounds[c * TPC + k + 1] - bounds[c * TPC + k])
                for c in range(CORES))
        nch_of_k.append(max(1, rup(m, P) // P))
    NPAIRS = sum(nch_of_k)
    NCH1 = max(nch_of_k) + 1      # +1 self chunk
    NCH2 = max(nch_of_k)
    G1COLS = sum((1 + nch) * P for nch in nch_of_k) // 16
    G2COLS = sum(nch * P for nch in nch_of_k) // 16

    # ---- per-core gather idx / dstl arrays ----
    in_maps = []
    iota = np.tile(np.arange(P, dtype=np.float32)[None, :], (P, 1))

    codes = (x[:, 0].astype(np.int64) * 50 + x[:, 1].astype(np.int64))
    codes_pad = np.zeros(NPAD, dtype=np.int64)
    codes_pad[:N] = codes
    # stage-1 idx list: 49 ops x (GS tiles + 1 pad chunk); same for all cores
    eidx_list = []
    for e in range(NTILES // GS):
        nodes = np.arange(e * GS * P, (e + 1) * GS * P)
        op = np.concatenate([codes_pad[nodes] - EMB_MID,
                             np.zeros(P, dtype=np.int64)])
        eidx_list.append(op)
    eidx = np.tile(wrap_idx(np.concatenate(eidx_list).astype(np.int16)), (8, 1))

    # host-side fused embedding table: emb_ab[a*50+b] = emb_a[a]@W1lo + emb_b[b]@W1mid
    a_part = np.asarray(emb_a, np.float32) @ np.asarray(W1, np.float32)[0:64]
    b_part = np.asarray(emb_b, np.float32) @ np.asarray(W1, np.float32)[64:128]
    emb_ab = (a_part[:, None, :] + b_part[None, :, :]).reshape(50000, C1)
    emb_ab = emb_ab.astype(ml_dtypes.bfloat16)

    xT = np.zeros((8, NPAD), dtype=np.float32)
    xT[:, :N] = x[:, 2:10].T
    xT = xT.astype(ml_dtypes.bfloat16)

    dinv_all = dinv.reshape(NTILES, P).T.copy()   # [P, NTILES]

    for c in range(CORES):
        g1 = np.zeros(G1COLS * 16, dtype=np.int64)
        g2 = np.zeros(G2COLS * 16, dtype=np.int64)
        dstlm = np.full((P, NPAIRS), PAD_DSTL, dtype=np.float32)
        o1 = o2 = 0
        pc = 0
        for k in range(TPC):
            nch = nch_of_k[k]
            t = core_tiles[c][k]
            # conv1 self chunk: own tile rows
            g1[o1:o1 + P] = t * P + np.arange(P) - T1_MID
            lo, hi = bounds[c * TPC + k], bounds[c * TPC + k + 1]
            m = int(hi - lo)
            i1 = np.zeros(nch * P, dtype=np.int64)
            i2 = np.zeros(nch * P, dtype=np.int64)
            dl = np.full(nch * P, PAD_DSTL, dtype=np.float32)
            i1[:m] = src_s[lo:hi] - T1_MID
            i2[:m] = trow2_s[lo:hi] - T2_MID
            dl[:m] = dstl_s[lo:hi]
            # ensure last wrapped element (list[-1]) is >= 0 in both lists
            if i1[-1] < 0 or i2[-1] < 0:
                ok = np.where((i1 >= 0) & (i2 >= 0))[0]
                assert len(ok), "no safe trailing idx in op"
                p_ = int(ok[0])
                for arr in (i1, i2, dl):
                    arr[p_], arr[-1] = arr[-1], arr[p_]
            g1[o1 + P:o1 + P + nch * P] = i1
            g2[o2:o2 + nch * P] = i2
            for j in range(nch):
                dstlm[:, pc + j] = dl[j * P:(j + 1) * P]
            o1 += (1 + nch) * P
            o2 += nch * P
            pc += nch
        assert o1 == G1COLS * 16 and o2 == G2COLS * 16 and pc == NPAIRS

        # self-chunk trailing check: self idx can be negative only if the
        # slot's op list ends with it -- never (edge chunks follow; nch>=1)
        gidx1 = np.tile(wrap_idx(g1.astype(np.int16)), (8, 1))
        gidx2 = np.tile(wrap_idx(g2.astype(np.int16)), (8, 1))

        nodes_own = np.concatenate(
            [t * P + np.arange(P) for t in core_tiles[c]])
        dinvk = dinv[nodes_own].reshape(TPC, P).T.copy()

        in_maps.append({
            "emb_ab": emb_ab,
            "xT": xT,
            "eidx": eidx.copy(),
            "gidx1": gidx1,
            "gidx2": gidx2,
            "dstlm": dstlm,
            "dinv_all": dinv_all,
            "dinvk": dinvk,
            "W1hi": np.asarray(W1)[128:136].astype(ml_dtypes.bfloat16),
            "W2": np.asarray(W2, dtype=np.float32),
            "b1f": np.tile(np.asarray(b1, np.float32)[None, :], (P, 1)),
            "b2f": np.tile(np.asarray(b2, np.float32)[None, :], (P, 1)),
            "iota": iota,
        })

    meta = {"nch_of_k": tuple(nch_of_k), "NPAIRS": NPAIRS, "NCH1": NCH1,
            "NCH2": NCH2, "G1COLS": G1COLS, "G2COLS": G2COLS,
            "core_tiles": core_tiles}
    return in_maps, meta


def build(meta):
    nch_of_k = meta["nch_of_k"]
    NPAIRS = meta["NPAIRS"]
    NCH1 = meta["NCH1"]
    NCH2 = meta["NCH2"]
    G1COLS = meta["G1COLS"]
    G2COLS = meta["G2COLS"]
    ECOLS = (NTILES // GS) * (GS + 1) * P // 16

    nc = bacc.Bacc("TRN2", target_bir_lowering=False, debug=False,
                   num_devices=CORES, num_swdge_queues=NQ)
    emb_ab = nc.dram_tensor("emb_ab", [50000, C1], bf16, kind="ExternalInput")
    xT = nc.dram_tensor("xT", [8, NPAD], bf16, kind="ExternalInput")
    eidx = nc.dram_tensor("eidx", [P, ECOLS], i16, kind="ExternalInput")
    gidx1 = nc.dram_tensor("gidx1", [P, G1COLS], i16, kind="ExternalInput")
    gidx2 = nc.dram_tensor("gidx2", [P, G2COLS], i16, kind="ExternalInput")
    dstlm = nc.dram_tensor("dstlm", [P, NPAIRS], f32, kind="ExternalInput")
    dinv_all = nc.dram_tensor("dinv_all", [P, NTILES], f32, kind="ExternalInput")
    dinvk = nc.dram_tensor("dinvk", [P, TPC], f32, kind="ExternalInput")
    W1hi = nc.dram_tensor("W1hi", [8, C1], bf16, kind="ExternalInput")
    W2 = nc.dram_tensor("W2", [C1, C2], f32, kind="ExternalInput")
    b1f = nc.dram_tensor("b1f", [P, C1], f32, kind="ExternalInput")
    b2f = nc.dram_tensor("b2f", [P, C2], f32, kind="ExternalInput")
    iota = nc.dram_tensor("iota", [P, P], f32, kind="ExternalInput")
    y = nc.dram_tensor("y", [NLOC, C2], f32, kind="ExternalOutput")

    with tile.TileContext(nc) as tc:
        with tc.tile_pool(name="const", bufs=1) as cpool, \
             tc.tile_pool(name="meta", bufs=1) as mpool, \
             tc.tile_pool(name="ge", bufs=3) as gepool, \
             tc.tile_pool(name="xt", bufs=2) as xtpool, \
             tc.tile_pool(name="he1", bufs=4) as he1pool, \
             tc.tile_pool(name="he2", bufs=4) as he2pool, \
             tc.tile_pool(name="sel", bufs=4) as spool, \
             tc.tile_pool(name="epi", bufs=3) as tpool, \
             tc.tile_pool(name="stash", bufs=1) as stpool, \
             tc.tile_pool(name="ptr", bufs=2, space="PSUM") as ptrp, \
             tc.tile_pool(name="pmm", bufs=2, space="PSUM") as pmmp, \
             tc.tile_pool(name="pacc", bufs=4, space="PSUM") as paccp, \
             tc.tile_pool(name="dram", bufs=1, space="DRAM") as dram:

            # ---------- constants ----------
            ident = cpool.tile([P, P], f32, tag="ident")
            make_identity(nc, ident[:])
            identb = cpool.tile([P, P], bf16, tag="identb")
            nc.vector.tensor_copy(out=identb[:], in_=ident[:])
            iota_t = cpool.tile([P, P], f32, tag="iota")
            nc.sync.dma_start(out=iota_t[:], in_=iota[:])
            W1hi_t = cpool.tile([8, C1], bf16, tag="w1hi")
            nc.sync.dma_start(out=W1hi_t[:], in_=W1hi[:])
            W2t = cpool.tile([C1, C2], f32, tag="w2")
            nc.sync.dma_start(out=W2t[:], in_=W2[:])
            b1t = cpool.tile([P, C1], f32, tag="b1")
            nc.sync.dma_start(out=b1t[:], in_=b1f[:])
            b2t = cpool.tile([P, C2], f32, tag="b2")
            nc.sync.dma_start(out=b2t[:], in_=b2f[:])
            dinvA = cpool.tile([P, NTILES], f32, tag="dinvA")
            nc.sync.dma_start(out=dinvA[:], in_=dinv_all[:])
            dinvK = cpool.tile([P, TPC], f32, tag="dinvK")
            nc.sync.dma_start(out=dinvK[:], in_=dinvk[:])
            eidx_t = mpool.tile([P, ECOLS], i16, tag="eidx")
            nc.sync.dma_start(out=eidx_t[:], in_=eidx[:])
            gidx1_t = mpool.tile([P, G1COLS], i16, tag="gidx1")
            nc.sync.dma_start(out=gidx1_t[:], in_=gidx1[:])
            gidx2_t = mpool.tile([P, G2COLS], i16, tag="gidx2")
            nc.sync.dma_start(out=gidx2_t[:], in_=gidx2[:])
            dstl_t = mpool.tile([P, NPAIRS], f32, tag="dstl")
            nc.sync.dma_start(out=dstl_t[:], in_=dstlm[:])
            h2stash = stpool.tile([P, TPC * C2], f32, tag="h2stash")

            table1 = dram.tile([NPAD, C1], bf16, tag="table1")
            ag2 = dram.tile([NLOC, C2], f32, tag="ag2")
            table2 = dram.tile([NPAD, C2], f32, tag="table2")

            gq = [0]

            def next_q():
                q = gq[0] % NQ
                gq[0] += 1
                return q

            # ---------- stage 1 (replicated): build full h1 table ----------
            for e in range(NTILES // GS):
                nidx = (GS + 1) * P
                ge = gepool.tile([P, (GS + 1) * P], bf16, tag="ge",
                                 name=f"ge_{e}")
                nc.gpsimd.dma_gather(
                    out_ap=ge[:].rearrange("p (n c) -> p n c", c=C1),
                    in_ap=emb_ab[EMB_MID:, :],
                    idxs_ap=eidx_t[:, e * nidx // 16:(e + 1) * nidx // 16],
                    num_idxs=nidx, num_idxs_reg=nidx, elem_size=C1,
                    single_packet=False, queue_num=next_q())
                xt_c = xtpool.tile([8, GS * P], bf16, tag="xt", name=f"xt_{e}")
                nc.sync.dma_start(out=xt_c[:],
                                  in_=xT[:, e * GS * P:(e + 1) * GS * P])
                for j in range(GS):
                    t = e * GS + j
                    ph = pmmp.tile([P, C1], f32, space="PSUM", tag="pmm",
                                   name=f"ph1_{t}")
                    nc.tensor.matmul(out=ph[:], lhsT=identb[:],
                                     rhs=ge[:, j * C1:(j + 1) * C1],
                                     start=True, stop=False)
                    nc.tensor.matmul(out=ph[:], lhsT=xt_c[:, j * P:(j + 1) * P],
                                     rhs=W1hi_t[:], start=False, stop=True)
                    h1s = tpool.tile([P, C1], bf16, tag="h1s", name=f"h1s_{t}")
                    if t % 2 == 0:
                        nc.scalar.activation(
                            out=h1s[:], in_=ph[:],
                            func=mybir.ActivationFunctionType.Copy,
                            scale=dinvA[:, t:t + 1])
                    else:
                        nc.vector.tensor_tensor(
                            out=h1s[:], in0=ph[:],
                            in1=dinvA[:, t:t + 1].to_broadcast([P, C1]),
                            op=mybir.AluOpType.mult)
                    nc.sync.dma_start(out=table1[t * P:(t + 1) * P, :],
                                      in_=h1s[:])

            tc.strict_bb_all_engine_barrier()

            # ---------- conv1 ----------
            o1 = 0
            pc = 0
            for k in range(TPC):
                nch = nch_of_k[k]
                nidx = (1 + nch) * P
                he = he1pool.tile([P, NCH1 * C1], bf16, tag="he1",
                                  name=f"he1_{k}")
                nc.gpsimd.dma_gather(
                    out_ap=he[:, 0:(1 + nch) * C1].rearrange(
                        "p (n c) -> p n c", c=C1),
                    in_ap=table1[T1_MID:, :],
                    idxs_ap=gidx1_t[:, o1 // 16:(o1 + nidx) // 16],
                    num_idxs=nidx, num_idxs_reg=nidx, elem_size=C1,
                    single_packet=False, queue_num=next_q())
                o1 += nidx
                pacc = paccp.tile([P, C1], f32, space="PSUM", tag="pacc",
                                  name=f"pacc1_{k}")
                nb = 4
                for j0 in range(0, nch, nb):
                    j1 = min(j0 + nb, nch)
                    S = spool.tile([P, nb * P], bf16, tag="S1",
                                   name=f"S1_{k}_{j0}")
                    nc.vector.tensor_tensor(
                        out=S[:, 0:(j1 - j0) * P].rearrange(
                            "p (n c) -> p n c", c=P),
                        in0=dstl_t[:, pc + j0:pc + j1].rearrange(
                            "p (n c) -> p n c", c=1).to_broadcast(
                            [P, j1 - j0, P]),
                        in1=iota_t[:].rearrange("p (n c) -> p n c", n=1).to_broadcast(
                            [P, j1 - j0, P]),
                        op=mybir.AluOpType.is_equal)
                    for j in range(j0, j1):
                        nc.tensor.matmul(
                            out=pacc[:], lhsT=S[:, (j - j0) * P:(j - j0 + 1) * P],
                            rhs=he[:, (1 + j) * C1:(2 + j) * C1],
                            start=(j == 0), stop=(j == nch - 1))
                pc += nch
                # epilogue: z = relu(dinv*(pacc + self) + b1)
                t1 = tpool.tile([P, C1], f32, tag="t1", name=f"t1_{k}")
                nc.vector.tensor_add(out=t1[:], in0=pacc[:], in1=he[:, 0:C1])
                t2 = tpool.tile([P, C1], f32, tag="t2", name=f"t2_{k}")
                nc.scalar.activation(out=t2[:], in_=t1[:],
                                     func=mybir.ActivationFunctionType.Copy,
                                     scale=dinvK[:, k:k + 1])
                t3 = tpool.tile([P, C1], f32, tag="t3", name=f"t3_{k}")
                nc.vector.tensor_add(out=t3[:], in0=t2[:], in1=b1t[:])
                t4 = tpool.tile([P, C1], f32, tag="t4", name=f"t4_{k}")
                nc.vector.tensor_scalar_max(out=t4[:], in0=t3[:], scalar1=0.0)
                # h2 = (t4 @ W2) * dinv
                ptr2 = ptrp.tile([P, P], f32, space="PSUM", tag="ptr",
                                 name=f"ptr2_{k}")
                nc.tensor.transpose(out=ptr2[:], in_=t4[:], identity=ident[:])
                hT = tpool.tile([P, P], f32, tag="hT", name=f"hT_{k}")
                nc.vector.tensor_copy(out=hT[:], in_=ptr2[:])
                ph2 = pmmp.tile([P, C2], f32, space="PSUM", tag="pmm",
                                name=f"ph2_{k}")
                nc.tensor.matmul(out=ph2[:], lhsT=hT[:], rhs=W2t[:],
                                 start=True, stop=True)
                nc.scalar.activation(out=h2stash[:, k * C2:(k + 1) * C2],
                                     in_=ph2[:],
                                     func=mybir.ActivationFunctionType.Copy,
                                     scale=dinvK[:, k:k + 1])
                nc.sync.dma_start(out=ag2[k * P:(k + 1) * P, :],
                                  in_=h2stash[:, k * C2:(k + 1) * C2])

            nc.gpsimd.collective_compute(
                "AllGather", mybir.AluOpType.bypass,
                replica_groups=[list(range(CORES))],
                ins=[ag2.opt()], outs=[table2.opt()])

            # ---------- conv2 ----------
            o2 = 0
            pc = 0
            for k in range(TPC):
                nch = nch_of_k[k]
                nidx = nch * P
                he = he2pool.tile([P, NCH2 * C2], f32, tag="he2",
                                  name=f"he2_{k}")
                nc.gpsimd.dma_gather(
                    out_ap=he[:, 0:nch * C2].rearrange(
                        "p (n c) -> p n c", c=C2),
                    in_ap=table2[T2_MID:, :],
                    idxs_ap=gidx2_t[:, o2 // 16:(o2 + nidx) // 16],
                    num_idxs=nidx, num_idxs_reg=nidx, elem_size=C2,
                    single_packet=False, queue_num=next_q())
                o2 += nidx
                pacc = paccp.tile([P, C2], f32, space="PSUM", tag="pacc",
                                  name=f"pacc2_{k}")
                nb = 4
                for j0 in range(0, nch, nb):
                    j1 = min(j0 + nb, nch)
                    S = spool.tile([P, nb * P], f32, tag="S2",
                                   name=f"S2_{k}_{j0}")
                    nc.vector.tensor_tensor(
                        out=S[:, 0:(j1 - j0) * P].rearrange(
                            "p (n c) -> p n c", c=P),
                        in0=dstl_t[:, pc + j0:pc + j1].rearrange(
                            "p (n c) -> p n c", c=1).to_broadcast(
                            [P, j1 - j0, P]),
                        in1=iota_t[:].rearrange("p (n c) -> p n c", n=1).to_broadcast(
                            [P, j1 - j0, P]),
                        op=mybir.AluOpType.is_equal)
                    for j in range(j0, j1):
                        nc.tensor.matmul(
                            out=pacc[:], lhsT=S[:, (j - j0) * P:(j - j0 + 1) * P],
                            rhs=he[:, j * C2:(j + 1) * C2],
                            start=(j == 0), stop=(j == nch - 1))
                pc += nch
                t1 = tpool.tile([P, C2], f32, tag="u1", name=f"u1_{k}")
                nc.vector.tensor_add(out=t1[:], in0=pacc[:],
                                     in1=h2stash[:, k * C2:(k + 1) * C2])
                t2 = tpool.tile([P, C2], f32, tag="u2", name=f"u2_{k}")
                nc.scalar.activation(out=t2[:], in_=t1[:],
                                     func=mybir.ActivationFunctionType.Copy,
                                     scale=dinvK[:, k:k + 1])
                t3 = tpool.tile([P, C2], f32, tag="u3", name=f"u3_{k}")
                nc.vector.tensor_add(out=t3[:], in0=t2[:], in1=b2t[:])
                t4 = tpool.tile([P, C2], f32, tag="u4", name=f"u4_{k}")
                nc.vector.tensor_scalar_max(out=t4[:], in0=t3[:], scalar1=0.0)
                nc.sync.dma_start(out=y[k * P:(k + 1) * P, :], in_=t4[:])

    nc.compile()
    return nc


_cache = {}


def kernel(x, edge_index, emb_a, emb_b, W1, b1, W2, b2):
    in_maps, meta = prep(x, edge_index, emb_a, emb_b, W1, b1, W2, b2)
    key = (meta["nch_of_k"], meta["NPAIRS"])
    if key not in _cache:
        _cache[key] = build(meta)
    nc = _cache[key]
    res = run_bass_kernel_spmd(nc, in_maps, core_ids=list(range(CORES)))
    out = np.zeros((N, C2), dtype=np.float32)
    for c in range(CORES):
        yc = res.results[c]["y"]
        nodes = np.concatenate(
            [t * P + np.arange(P) for t in meta["core_tiles"][c]])
        valid = nodes < N
        out[nodes[valid]] = yc[valid]
    return out


# revision 5
# speedup vs baseline: 1.7205x; 1.3890x over previous
"""Self-contained GCN encoder kernel for 8 TRN2 NeuronCores (Bass/Tile).

kernel(**inputs) takes the FULL unsharded inputs (as from setup_inputs())
and returns the FULL [50000, 64] float32 output.

Strategy: stage 1 (embedding + W1) is REPLICATED on every core via a fused
host-precomputed lookup table emb_ab = emb_a@W1[:64] (+) emb_b@W1[64:128]
(one dma_gather per 8-tile group, accumulated into PSUM with an
identity-matmul, plus the numeric-feature matmul), writing the full
dinv-scaled h1 table to local DRAM -- no first AllGather, so the slow
startup CC barrier overlaps compute.  Conv aggregations shard dst-node
tiles across cores (LPT-balanced, quantile-matched slot order keeps the
SPMD stream identical); per-edge rows are fetched with dma_gather striped
over 4 SWDGE queues (4x descriptor-generation throughput) using a
mid-table base pointer and signed int16 indices (no A/B table split).
Seg-reduction is one-hot (is_equal) S-matrices x gathered rows on the
TensorEngine into PSUM; the symmetric norm is folded into table rows (src)
and the epilogue scale (dst); conv1 self-loop rows ride along as an extra
gather chunk, conv2 self-loop terms are stashed in SBUF from the conv1
epilogue.  One AllGather (h2 table) runs between the convs.
"""
import numpy as np
from concourse import bacc, mybir, tile
from concourse.bass_utils import run_bass_kernel_spmd
from concourse.masks import make_identity

P = 128
CORES = 8
N = 50000
NTILES = 392
NPAD = NTILES * P      # 50176
TPC = NTILES // CORES  # 49
NLOC = TPC * P         # 6272
C1 = 128
C2 = 64
EMB_MID = 25000
T1_MID = NPAD // 2     # 25088
T2_MID = NPAD // 2
PAD_DSTL = 30000.0
GS = 8                 # tiles per stage-1 gather op
NQ = 4                 # SWDGE queues

f32 = mybir.dt.float32
bf16 = mybir.dt.bfloat16
i16 = mybir.dt.int16


def wrap_idx(arr):
    return arr.reshape(-1, 16).T


def rup(x, m):
    return int((x + m - 1) // m * m)


def prep(x, edge_index, emb_a, emb_b, W1, b1, W2, b2):
    import ml_dtypes
    x = np.asarray(x)
    src, dst = np.asarray(edge_index[0]).astype(np.int64), \
        np.asarray(edge_index[1]).astype(np.int64)
    deg = np.bincount(dst, minlength=N).astype(np.float32) + 1.0
    dinv = np.ones(NPAD, dtype=np.float32)
    dinv[:N] = 1.0 / np.sqrt(deg)

    # ---- tile -> core assignment (LPT on edge counts) ----
    t_of_e = dst // P
    tile_cnt = np.bincount(t_of_e, minlength=NTILES)
    order = np.argsort(-tile_cnt, kind="stable")
    core_loads = np.zeros(CORES, dtype=np.int64)
    core_tiles = [[] for _ in range(CORES)]
    for t in order:
        c = int(np.argmin(core_loads))
        core_tiles[c].append(int(t))
        core_loads[c] += tile_cnt[t]
    c_of_t = np.zeros(NTILES, dtype=np.int64)
    k_of_t = np.zeros(NTILES, dtype=np.int64)
    for c in range(CORES):
        for k, t in enumerate(core_tiles[c]):
            c_of_t[t] = c
            k_of_t[t] = k

    node_ids = np.arange(NPAD)
    trow2 = c_of_t[node_ids // P] * NLOC + k_of_t[node_ids // P] * P \
        + node_ids % P

    # ---- sort edges by (core, slot) ----
    key = c_of_t[t_of_e] * TPC + k_of_t[t_of_e]
    sort = np.argsort(key, kind="stable")
    src_s = src[sort]
    trow2_s = trow2[src_s]
    dstl_s = (dst % P).astype(np.float32)[sort]
    bounds = np.searchsorted(key[sort], np.arange(CORES * TPC + 1))

    # ---- op schedule: one op per slot k; nch = max over cores ----
    nch_of_k = []
    for k in range(TPC):
        m = max(int(bounds[c * TPC + k + 1] - bounds[c * TPC + k])
                for c in range(CORES))
        nch_of_k.append(max(1, rup(m, P) // P))
    NPAIRS = sum(nch_of_k)
    NCH1 = max(nch_of_k) + 1      # +1 self chunk
    NCH2 = max(nch_of_k)
    G1COLS = sum((1 + nch) * P for nch in nch_of_k) // 16
    G2COLS = sum(nch * P for nch in nch_of_k) // 16

    # ---- per-core gather idx / dstl arrays ----
    in_maps = []
    iota = np.tile(np.arange(P, dtype=np.float32)[None, :], (P, 1))

    codes = (x[:, 0].astype(np.int64) * 50 + x[:, 1].astype(np.int64))
    codes_pad = np.zeros(NPAD, dtype=np.int64)
    codes_pad[:N] = codes
    # stage-1 idx list: 49 ops x (GS tiles + 1 pad chunk); same for all cores
    eidx_list = []
    for e in range(NTILES // GS):
        nodes = np.arange(e * GS * P, (e + 1) * GS * P)
        op = np.concatenate([codes_pad[nodes] - EMB_MID,
                             np.zeros(P, dtype=np.int64)])
        eidx_list.append(op)
    eidx = np.tile(wrap_idx(np.concatenate(eidx_list).astype(np.int16)), (8, 1))

    # host-side fused embedding table: emb_ab[a*50+b] = emb_a[a]@W1lo + emb_b[b]@W1mid
    a_part = np.asarray(emb_a, np.float32) @ np.asarray(W1, np.float32)[0:64]
    b_part = np.asarray(emb_b, np.float32) @ np.asarray(W1, np.float32)[64:128]
    emb_ab = (a_part[:, None, :] + b_part[None, :, :]).reshape(50000, C1)
    emb_ab = emb_ab.astype(ml_dtypes.bfloat16)

    xT = np.zeros((8, NPAD), dtype=np.float32)
    xT[:, :N] = x[:, 2:10].T
    xT = xT.astype(ml_dtypes.bfloat16)

    dinv_all = dinv.reshape(NTILES, P).T.copy()   # [P, NTILES]

    for c in range(CORES):
        g1 = np.zeros(G1COLS * 16, dtype=np.int64)
        g2 = np.zeros(G2COLS * 16, dtype=np.int64)
        dstlm = np.full((P, NPAIRS), PAD_DSTL, dtype=np.float32)
        o1 = o2 = 0
        pc = 0
        for k in range(TPC):
            nch = nch_of_k[k]
            t = core_tiles[c][k]
            # conv1 self chunk: own tile rows
            g1[o1:o1 + P] = t * P + np.arange(P) - T1_MID
            lo, hi = bounds[c * TPC + k], bounds[c * TPC + k + 1]
            m = int(hi - lo)
            i1 = np.zeros(nch * P, dtype=np.int64)
            i2 = np.zeros(nch * P, dtype=np.int64)
            dl = np.full(nch * P, PAD_DSTL, dtype=np.float32)
            i1[:m] = src_s[lo:hi] - T1_MID
            i2[:m] = trow2_s[lo:hi] - T2_MID
            dl[:m] = dstl_s[lo:hi]
            # ensure last wrapped element (list[-1]) is >= 0 in both lists
            if i1[-1] < 0 or i2[-1] < 0:
                ok = np.where((i1 >= 0) & (i2 >= 0))[0]
                assert len(ok), "no safe trailing idx in op"
                p_ = int(ok[0])
                for arr in (i1, i2, dl):
                    arr[p_], arr[-1] = arr[-1], arr[p_]
            g1[o1 + P:o1 + P + nch * P] = i1
            g2[o2:o2 + nch * P] = i2
            for j in range(nch):
                dstlm[:, pc + j] = dl[j * P:(j + 1) * P]
            o1 += (1 + nch) * P
            o2 += nch * P
            pc += nch
        assert o1 == G1COLS * 16 and o2 == G2COLS * 16 and pc == NPAIRS

        # self-chunk trailing check: self idx can be negative only if the
        # slot's op list ends with it -- never (edge chunks follow; nch>=1)
        gidx1 = np.tile(wrap_idx(g1.astype(np.int16)), (8, 1))
        gidx2 = np.tile(wrap_idx(g2.astype(np.int16)), (8, 1))

        nodes_own = np.concatenate(
            [t * P + np.arange(P) for t in core_tiles[c]])
        dinvk = dinv[nodes_own].reshape(TPC, P).T.copy()

        rdk = (1.0 / dinvk).reshape(1, -1, order="F").astype(np.float32)

        in_maps.append({
            "emb_ab": emb_ab,
            "xT": xT,
            "eidx": eidx.copy(),
            "gidx1": gidx1,
            "gidx2": gidx2,
            "dstlm": dstlm.astype(ml_dtypes.bfloat16), "dstlf": dstlm,
            "dinv_all": dinv_all,
            "dinvk": dinvk,
            "W1hi": np.asarray(W1)[128:136].astype(ml_dtypes.bfloat16),
            "W2": np.asarray(W2, dtype=np.float32),
            "b1f": np.tile(np.asarray(b1, np.float32)[None, :], (P, 1)),
            "b2f": np.tile(np.asarray(b2, np.float32)[None, :], (P, 1)),
            "iota": iota, "iotab": iota.astype(ml_dtypes.bfloat16), "rdk": rdk,
        })

    meta = {"nch_of_k": tuple(nch_of_k), "NPAIRS": NPAIRS, "NCH1": NCH1,
            "NCH2": NCH2, "G1COLS": G1COLS, "G2COLS": G2COLS,
            "core_tiles": core_tiles}
    return in_maps, meta


def build(meta):
    nch_of_k = meta["nch_of_k"]
    NPAIRS = meta["NPAIRS"]
    NCH1 = meta["NCH1"]
    NCH2 = meta["NCH2"]
    G1COLS = meta["G1COLS"]
    G2COLS = meta["G2COLS"]
    ECOLS = (NTILES // GS) * (GS + 1) * P // 16

    nc = bacc.Bacc("TRN2", target_bir_lowering=False, debug=False,
                   num_devices=CORES, num_swdge_queues=NQ)
    emb_ab = nc.dram_tensor("emb_ab", [50000, C1], bf16, kind="ExternalInput")
    xT = nc.dram_tensor("xT", [8, NPAD], bf16, kind="ExternalInput")
    eidx = nc.dram_tensor("eidx", [P, ECOLS], i16, kind="ExternalInput")
    gidx1 = nc.dram_tensor("gidx1", [P, G1COLS], i16, kind="ExternalInput")
    gidx2 = nc.dram_tensor("gidx2", [P, G2COLS], i16, kind="ExternalInput")
    dstlm = nc.dram_tensor("dstlm", [P, NPAIRS], bf16, kind="ExternalInput")
    iotab = nc.dram_tensor("iotab", [P, P], bf16, kind="ExternalInput")
    dstlf = nc.dram_tensor("dstlf", [P, NPAIRS], f32, kind="ExternalInput")
    rdk = nc.dram_tensor("rdk", [1, NLOC], f32, kind="ExternalInput")
    dinv_all = nc.dram_tensor("dinv_all", [P, NTILES], f32, kind="ExternalInput")
    dinvk = nc.dram_tensor("dinvk", [P, TPC], f32, kind="ExternalInput")
    W1hi = nc.dram_tensor("W1hi", [8, C1], bf16, kind="ExternalInput")
    W2 = nc.dram_tensor("W2", [C1, C2], f32, kind="ExternalInput")
    b1f = nc.dram_tensor("b1f", [P, C1], f32, kind="ExternalInput")
    b2f = nc.dram_tensor("b2f", [P, C2], f32, kind="ExternalInput")
    iota = nc.dram_tensor("iota", [P, P], f32, kind="ExternalInput")
    y = nc.dram_tensor("y", [NLOC, C2], f32, kind="ExternalOutput")

    with tile.TileContext(nc) as tc:
        with tc.tile_pool(name="const", bufs=1) as cpool, \
             tc.tile_pool(name="meta", bufs=1) as mpool, \
             tc.tile_pool(name="ge", bufs=8) as gepool, \
             tc.tile_pool(name="xt", bufs=4) as xtpool, \
             tc.tile_pool(name="he1", bufs=8) as he1pool, \
             tc.tile_pool(name="he2", bufs=8) as he2pool, \
             tc.tile_pool(name="sel", bufs=8) as spool, \
             tc.tile_pool(name="epi", bufs=3) as tpool, \
             tc.tile_pool(name="stash", bufs=1) as stpool, \
             tc.tile_pool(name="ptr", bufs=1, space="PSUM") as ptrp, \
             tc.tile_pool(name="pmm", bufs=2, space="PSUM") as pmmp, \
             tc.tile_pool(name="pacc", bufs=5, space="PSUM") as paccp, \
             tc.tile_pool(name="dram", bufs=1, space="DRAM") as dram:

            # ---------- constants ----------
            ident = cpool.tile([P, P], f32, tag="ident")
            make_identity(nc, ident[:])
            identb = cpool.tile([P, P], bf16, tag="identb")
            nc.vector.tensor_copy(out=identb[:], in_=ident[:])
            iota_t = cpool.tile([P, P], f32, tag="iota")
            nc.sync.dma_start(out=iota_t[:], in_=iota[:])
            iotab_t = cpool.tile([P, P], bf16, tag="iotab")
            nc.sync.dma_start(out=iotab_t[:], in_=iotab[:])
            rdk_t = cpool.tile([1, NLOC], f32, tag="rdk")
            nc.sync.dma_start(out=rdk_t[:], in_=rdk[:])
            W1hi_t = cpool.tile([8, C1], bf16, tag="w1hi")
            nc.sync.dma_start(out=W1hi_t[:], in_=W1hi[:])
            W2t = cpool.tile([C1, C2], f32, tag="w2")
            nc.sync.dma_start(out=W2t[:], in_=W2[:])
            b1t = cpool.tile([P, C1], f32, tag="b1")
            nc.sync.dma_start(out=b1t[:], in_=b1f[:])
            b2t = cpool.tile([P, C2], f32, tag="b2")
            nc.sync.dma_start(out=b2t[:], in_=b2f[:])
            dinvA = cpool.tile([P, NTILES], f32, tag="dinvA")
            nc.sync.dma_start(out=dinvA[:], in_=dinv_all[:])
            dinvK = cpool.tile([P, TPC], f32, tag="dinvK")
            nc.sync.dma_start(out=dinvK[:], in_=dinvk[:])
            eidx_t = mpool.tile([P, ECOLS], i16, tag="eidx")
            nc.sync.dma_start(out=eidx_t[:], in_=eidx[:])
            gidx1_t = mpool.tile([P, G1COLS], i16, tag="gidx1")
            nc.sync.dma_start(out=gidx1_t[:], in_=gidx1[:])
            gidx2_t = mpool.tile([P, G2COLS], i16, tag="gidx2")
            nc.sync.dma_start(out=gidx2_t[:], in_=gidx2[:])
            dstl_t = mpool.tile([P, NPAIRS], bf16, tag="dstl")
            nc.sync.dma_start(out=dstl_t[:], in_=dstlm[:])
            dstlf_t = mpool.tile([P, NPAIRS], f32, tag="dstlf")
            nc.sync.dma_start(out=dstlf_t[:], in_=dstlf[:])
            h2stash = stpool.tile([P, TPC * C2], f32, tag="h2stash")

            table1 = dram.tile([NPAD, C1], bf16, tag="table1")
            ag2 = dram.tile([NLOC, C2], f32, tag="ag2")
            table2 = dram.tile([NPAD, C2], f32, tag="table2")

            gq = [0]

            def next_q():
                q = gq[0] % NQ
                gq[0] += 1
                return q

            # ---------- stage 1 (replicated): build full h1 table ----------
            for e in range(NTILES // GS):
                nidx = (GS + 1) * P
                ge = gepool.tile([P, (GS + 1) * P], bf16, tag="ge",
                                 name=f"ge_{e}")
                nc.gpsimd.dma_gather(
                    out_ap=ge[:].rearrange("p (n c) -> p n c", c=C1),
                    in_ap=emb_ab[EMB_MID:, :],
                    idxs_ap=eidx_t[:, e * nidx // 16:(e + 1) * nidx // 16],
                    num_idxs=nidx, num_idxs_reg=nidx, elem_size=C1,
                    single_packet=False, queue_num=next_q())
                xt_c = xtpool.tile([8, GS * P], bf16, tag="xt", name=f"xt_{e}")
                nc.sync.dma_start(out=xt_c[:],
                                  in_=xT[:, e * GS * P:(e + 1) * GS * P])
                for j in range(GS):
                    t = e * GS + j
                    php = pmmp if j % 2 == 0 else paccp
                    ph = php.tile([P, C1], f32, space="PSUM",
                                  tag="pmm" if j % 2 == 0 else "pacc",
                                  name=f"ph1_{t}")
                    nc.tensor.matmul(out=ph[:], lhsT=identb[:],
                                     rhs=ge[:, j * C1:(j + 1) * C1],
                                     start=True, stop=False)
                    nc.tensor.matmul(out=ph[:], lhsT=xt_c[:, j * P:(j + 1) * P],
                                     rhs=W1hi_t[:], start=False, stop=True)
                    h1s = tpool.tile([P, C1], bf16, tag="h1s", name=f"h1s_{t}")
                    if t % 2 == 0:
                        nc.scalar.activation(
                            out=h1s[:], in_=ph[:],
                            func=mybir.ActivationFunctionType.Copy,
                            scale=dinvA[:, t:t + 1])
                    else:
                        nc.vector.tensor_tensor(
                            out=h1s[:], in0=ph[:],
                            in1=dinvA[:, t:t + 1].to_broadcast([P, C1]),
                            op=mybir.AluOpType.mult)
                    nc.sync.dma_start(out=table1[t * P:(t + 1) * P, :],
                                      in_=h1s[:])

            tc.strict_bb_all_engine_barrier()

            # ---------- conv1 ----------
            o1 = 0
            pc = 0
            for k in range(TPC):
                nch = nch_of_k[k]
                nidx = (1 + nch) * P
                he = he1pool.tile([P, NCH1 * C1], bf16, tag="he1",
                                  name=f"he1_{k}")
                nc.gpsimd.dma_gather(
                    out_ap=he[:, 0:(1 + nch) * C1].rearrange(
                        "p (n c) -> p n c", c=C1),
                    in_ap=table1[T1_MID:, :],
                    idxs_ap=gidx1_t[:, o1 // 16:(o1 + nidx) // 16],
                    num_idxs=nidx, num_idxs_reg=nidx, elem_size=C1,
                    single_packet=False, queue_num=next_q())
                o1 += nidx
                pacc = paccp.tile([P, C1], f32, space="PSUM", tag="pacc",
                                  name=f"pacc1_{k}")
                for j in range(nch):
                    S = spool.tile([P, P], bf16, tag="S1", name=f"S1_{k}_{j}")
                    nc.vector.tensor_tensor(
                        out=S[:],
                        in0=dstl_t[:, pc + j:pc + j + 1].to_broadcast([P, P]),
                        in1=iotab_t[:], op=mybir.AluOpType.is_equal)
                    nc.tensor.matmul(
                        out=pacc[:], lhsT=S[:],
                        rhs=he[:, (1 + j) * C1:(2 + j) * C1],
                        start=(j == 0), stop=False)
                pc += nch
                # pacc += self rows; pacc += b1/dinv (so relu(dinv*pacc) is exact)
                nc.tensor.matmul(out=pacc[:], lhsT=identb[:], rhs=he[:, 0:C1],
                                 start=False, stop=False)
                nc.tensor.matmul(out=pacc[:], lhsT=rdk_t[:, k * P:(k + 1) * P],
                                 rhs=b1t[0:1, :], start=False, stop=True)
                t4 = tpool.tile([P, C1], f32, tag="t4", name=f"t4_{k}")
                nc.scalar.activation(out=t4[:], in_=pacc[:],
                                     func=mybir.ActivationFunctionType.Relu,
                                     scale=dinvK[:, k:k + 1])
                # h2 = (t4 @ W2) * dinv
                ptr2 = ptrp.tile([P, P], f32, space="PSUM", tag="ptr",
                                 name=f"ptr2_{k}")
                nc.tensor.transpose(out=ptr2[:], in_=t4[:], identity=ident[:])
                hT = tpool.tile([P, P], f32, tag="hT", name=f"hT_{k}")
                nc.vector.tensor_copy(out=hT[:], in_=ptr2[:])
                ph2 = pmmp.tile([P, C2], f32, space="PSUM", tag="pmm",
                                name=f"ph2_{k}")
                nc.tensor.matmul(out=ph2[:], lhsT=hT[:], rhs=W2t[:],
                                 start=True, stop=True)
                nc.scalar.activation(out=h2stash[:, k * C2:(k + 1) * C2],
                                     in_=ph2[:],
                                     func=mybir.ActivationFunctionType.Copy,
                                     scale=dinvK[:, k:k + 1])
                nc.sync.dma_start(out=ag2[k * P:(k + 1) * P, :],
                                  in_=h2stash[:, k * C2:(k + 1) * C2])

            nc.gpsimd.collective_compute(
                "AllGather", mybir.AluOpType.bypass,
                replica_groups=[list(range(CORES))],
                ins=[ag2.opt()], outs=[table2.opt()])

            # ---------- conv2 ----------
            o2 = 0
            pc = 0
            for k in range(TPC):
                nch = nch_of_k[k]
                nidx = nch * P
                he = he2pool.tile([P, NCH2 * C2], f32, tag="he2",
                                  name=f"he2_{k}")
                nc.gpsimd.dma_gather(
                    out_ap=he[:, 0:nch * C2].rearrange(
                        "p (n c) -> p n c", c=C2),
                    in_ap=table2[T2_MID:, :],
                    idxs_ap=gidx2_t[:, o2 // 16:(o2 + nidx) // 16],
                    num_idxs=nidx, num_idxs_reg=nidx, elem_size=C2,
                    single_packet=False, queue_num=next_q())
                o2 += nidx
                pacc = paccp.tile([P, C2], f32, space="PSUM", tag="pacc",
                                  name=f"pacc2_{k}")
                for j in range(nch):
                    S = spool.tile([P, P], f32, tag="S2", name=f"S2_{k}_{j}")
                    nc.vector.tensor_tensor(
                        out=S[:],
                        in0=dstlf_t[:, pc + j:pc + j + 1].to_broadcast([P, P]),
                        in1=iota_t[:], op=mybir.AluOpType.is_equal)
                    nc.tensor.matmul(
                        out=pacc[:], lhsT=S[:],
                        rhs=he[:, j * C2:(j + 1) * C2],
                        start=(j == 0), stop=False)
                pc += nch
                nc.tensor.matmul(out=pacc[:], lhsT=ident[:],
                                 rhs=h2stash[:, k * C2:(k + 1) * C2],
                                 start=False, stop=False)
                nc.tensor.matmul(out=pacc[:], lhsT=rdk_t[:, k * P:(k + 1) * P],
                                 rhs=b2t[0:1, :], start=False, stop=True)
                t4 = tpool.tile([P, C2], f32, tag="u4", name=f"u4_{k}")
                nc.scalar.activation(out=t4[:], in_=pacc[:],
                                     func=mybir.ActivationFunctionType.Relu,
                                     scale=dinvK[:, k:k + 1])
                nc.sync.dma_start(out=y[k * P:(k + 1) * P, :], in_=t4[:])

    nc.compile()
    return nc


_cache = {}


def kernel(x, edge_index, emb_a, emb_b, W1, b1, W2, b2):
    in_maps, meta = prep(x, edge_index, emb_a, emb_b, W1, b1, W2, b2)
    key = (meta["nch_of_k"], meta["NPAIRS"])
    if key not in _cache:
        _cache[key] = build(meta)
    nc = _cache[key]
    res = run_bass_kernel_spmd(nc, in_maps, core_ids=list(range(CORES)))
    out = np.zeros((N, C2), dtype=np.float32)
    for c in range(CORES):
        yc = res.results[c]["y"]
        nodes = np.concatenate(
            [t * P + np.arange(P) for t in meta["core_tiles"][c]])
        valid = nodes < N
        out[nodes[valid]] = yc[valid]
    return out


# revision 8
# speedup vs baseline: 1.9991x; 1.1620x over previous
"""Self-contained GCN encoder kernel for 8 TRN2 NeuronCores (Bass/Tile).

kernel(**inputs) takes the FULL unsharded inputs (as from setup_inputs())
and returns the FULL [50000, 64] float32 output.

Strategy: stage 1 (embedding + W1) is REPLICATED on every core via a fused
host-precomputed lookup table emb_ab = emb_a@W1[:64] (+) emb_b@W1[64:128]
(one dma_gather per 8-tile group, accumulated into PSUM with an
identity-matmul, plus the numeric-feature matmul), writing the full
dinv-scaled h1 table to local DRAM -- no first AllGather, so the slow
startup CC barrier overlaps compute.  Conv aggregations shard dst-node
tiles across cores (LPT-balanced, quantile-matched slot order keeps the
SPMD stream identical); per-edge rows are fetched with dma_gather striped
over 4 SWDGE queues (4x descriptor-generation throughput) using a
mid-table base pointer and signed int16 indices (no A/B table split).
Seg-reduction is one-hot (is_equal) S-matrices x gathered rows on the
TensorEngine into PSUM; the symmetric norm is folded into table rows (src)
and the epilogue scale (dst); conv1 self-loop rows ride along as an extra
gather chunk, conv2 self-loop terms are stashed in SBUF from the conv1
epilogue.  One AllGather (h2 table) runs between the convs.
"""
import numpy as np
from concourse import bacc, mybir, tile
from concourse.bass_utils import run_bass_kernel_spmd
from concourse.masks import make_identity

P = 128
CORES = 8
N = 50000
NTILES = 392
NPAD = NTILES * P      # 50176
TPC = NTILES // CORES  # 49
NLOC = TPC * P         # 6272
C1 = 128
C2 = 64
EMB_MID = 25000
T1_MID = NPAD // 2     # 25088
T2_MID = NPAD // 2
PAD_DSTL = 30000.0
GS = 8                 # tiles per stage-1 gather op
NQ = 4                 # SWDGE queues

f32 = mybir.dt.float32
bf16 = mybir.dt.bfloat16
i16 = mybir.dt.int16


def wrap_idx(arr):
    return arr.reshape(-1, 16).T


def rup(x, m):
    return int((x + m - 1) // m * m)


def prep(x, edge_index, emb_a, emb_b, W1, b1, W2, b2):
    import ml_dtypes
    x = np.asarray(x)
    src, dst = np.asarray(edge_index[0]).astype(np.int64), \
        np.asarray(edge_index[1]).astype(np.int64)
    deg = np.bincount(dst, minlength=N).astype(np.float32) + 1.0
    dinv = np.ones(NPAD, dtype=np.float32)
    dinv[:N] = 1.0 / np.sqrt(deg)

    # ---- tile -> core assignment (LPT on edge counts) ----
    t_of_e = dst // P
    tile_cnt = np.bincount(t_of_e, minlength=NTILES)
    order = np.argsort(-tile_cnt, kind="stable")
    core_loads = np.zeros(CORES, dtype=np.int64)
    core_tiles = [[] for _ in range(CORES)]
    for t in order:
        c = int(np.argmin(core_loads))
        core_tiles[c].append(int(t))
        core_loads[c] += tile_cnt[t]
    c_of_t = np.zeros(NTILES, dtype=np.int64)
    k_of_t = np.zeros(NTILES, dtype=np.int64)
    for c in range(CORES):
        for k, t in enumerate(core_tiles[c]):
            c_of_t[t] = c
            k_of_t[t] = k

    node_ids = np.arange(NPAD)
    trow2 = c_of_t[node_ids // P] * NLOC + k_of_t[node_ids // P] * P \
        + node_ids % P

    # ---- sort edges by (core, slot) ----
    key = c_of_t[t_of_e] * TPC + k_of_t[t_of_e]
    sort = np.argsort(key, kind="stable")
    src_s = src[sort]
    trow2_s = trow2[src_s]
    dstl_s = (dst % P).astype(np.float32)[sort]
    bounds = np.searchsorted(key[sort], np.arange(CORES * TPC + 1))

    # ---- op schedule: one op per slot k; nch = max over cores ----
    nch_of_k = []
    for k in range(TPC):
        m = max(int(bounds[c * TPC + k + 1] - bounds[c * TPC + k])
                for c in range(CORES))
        nch_of_k.append(max(1, rup(m, P) // P))
    NPAIRS = sum(nch_of_k)
    NCH1 = max(nch_of_k) + 1      # +1 self chunk
    NCH2 = max(nch_of_k)
    G1COLS = sum((1 + nch) * P for nch in nch_of_k) // 16
    G2COLS = sum(nch * P for nch in nch_of_k) // 16

    # ---- per-core gather idx / dstl arrays ----
    in_maps = []
    iota = np.tile(np.arange(P, dtype=np.float32)[None, :], (P, 1))

    codes_a = np.zeros(NPAD, dtype=np.int64)
    codes_a[:N] = x[:, 0].astype(np.int64)
    codes_b = np.zeros(NPAD, dtype=np.int64)
    codes_b[:N] = x[:, 1].astype(np.int64)
    # stage-1 idx list: 49 ops x GS tiles; idx = cat_a (>=0, no trailing issue)
    eidx = np.tile(wrap_idx(codes_a.astype(np.int16)), (8, 1))

    # small gather table: emb_a@W1lo [1000, 128]; emb_b part via one-hot matmul
    emb_aw = (np.asarray(emb_a, np.float32)
              @ np.asarray(W1, np.float32)[0:64]).astype(ml_dtypes.bfloat16)
    W1Bp = (np.asarray(emb_b, np.float32)
            @ np.asarray(W1, np.float32)[64:128]).astype(ml_dtypes.bfloat16)
    xbT_rep = np.tile(codes_b.astype(np.float32)[None, :],
                      (50, 1)).astype(ml_dtypes.bfloat16)
    iotap = np.arange(P, dtype=np.float32)[:, None].astype(ml_dtypes.bfloat16)

    xT = np.zeros((8, NPAD), dtype=np.float32)
    xT[:, :N] = x[:, 2:10].T
    xT = xT.astype(ml_dtypes.bfloat16)

    dinv_all = dinv.reshape(NTILES, P).T.copy()   # [P, NTILES]

    for c in range(CORES):
        g1 = np.zeros(G1COLS * 16, dtype=np.int64)
        g2 = np.zeros(G2COLS * 16, dtype=np.int64)
        dstlm = np.full((P, NPAIRS), PAD_DSTL, dtype=np.float32)
        o1 = o2 = 0
        pc = 0
        for k in range(TPC):
            nch = nch_of_k[k]
            t = core_tiles[c][k]
            # conv1 self chunk: own tile rows
            g1[o1:o1 + P] = t * P + np.arange(P) - T1_MID
            lo, hi = bounds[c * TPC + k], bounds[c * TPC + k + 1]
            m = int(hi - lo)
            i1 = np.zeros(nch * P, dtype=np.int64)
            i2 = np.zeros(nch * P, dtype=np.int64)
            dl = np.full(nch * P, PAD_DSTL, dtype=np.float32)
            i1[:m] = src_s[lo:hi] - T1_MID
            i2[:m] = trow2_s[lo:hi] - T2_MID
            dl[:m] = dstl_s[lo:hi]
            # ensure last wrapped element (list[-1]) is >= 0 in both lists
            if i1[-1] < 0 or i2[-1] < 0:
                ok = np.where((i1 >= 0) & (i2 >= 0))[0]
                assert len(ok), "no safe trailing idx in op"
                p_ = int(ok[0])
                for arr in (i1, i2, dl):
                    arr[p_], arr[-1] = arr[-1], arr[p_]
            g1[o1 + P:o1 + P + nch * P] = i1
            g2[o2:o2 + nch * P] = i2
            for j in range(nch):
                dstlm[:, pc + j] = dl[j * P:(j + 1) * P]
            o1 += (1 + nch) * P
            o2 += nch * P
            pc += nch
        assert o1 == G1COLS * 16 and o2 == G2COLS * 16 and pc == NPAIRS

        # self-chunk trailing check: self idx can be negative only if the
        # slot's op list ends with it -- never (edge chunks follow; nch>=1)
        gidx1 = np.tile(wrap_idx(g1.astype(np.int16)), (8, 1))
        gidx2 = np.tile(wrap_idx(g2.astype(np.int16)), (8, 1))

        nodes_own = np.concatenate(
            [t * P + np.arange(P) for t in core_tiles[c]])
        dinvk = dinv[nodes_own].reshape(TPC, P).T.copy()

        rdk = (1.0 / dinvk).reshape(1, -1, order="F").astype(np.float32)

        in_maps.append({
            "emb_aw": emb_aw, "W1Bp": W1Bp, "xbT_rep": xbT_rep,
            "iotap": iotap,
            "xT": xT,
            "eidx": eidx.copy(),
            "gidx1": gidx1,
            "gidx2": gidx2,
            "dstlm": dstlm.astype(ml_dtypes.bfloat16), "dstlf": dstlm,
            "dinv_all": dinv_all,
            "dinvk": dinvk,
            "W1hi": np.asarray(W1)[128:136].astype(ml_dtypes.bfloat16),
            "W2": np.asarray(W2, dtype=np.float32),
            "b1f": np.tile(np.asarray(b1, np.float32)[None, :], (P, 1)),
            "b2f": np.tile(np.asarray(b2, np.float32)[None, :], (P, 1)),
            "iota": iota, "iotab": iota.astype(ml_dtypes.bfloat16), "rdk": rdk,
        })

    meta = {"nch_of_k": tuple(nch_of_k), "NPAIRS": NPAIRS, "NCH1": NCH1,
            "NCH2": NCH2, "G1COLS": G1COLS, "G2COLS": G2COLS,
            "core_tiles": core_tiles}
    return in_maps, meta


def build(meta):
    nch_of_k = meta["nch_of_k"]
    NPAIRS = meta["NPAIRS"]
    NCH1 = meta["NCH1"]
    NCH2 = meta["NCH2"]
    G1COLS = meta["G1COLS"]
    G2COLS = meta["G2COLS"]
    ECOLS = NTILES * P // 16

    nc = bacc.Bacc("TRN2", target_bir_lowering=False, debug=False,
                   num_devices=CORES, num_swdge_queues=NQ)
    emb_aw = nc.dram_tensor("emb_aw", [1000, C1], bf16, kind="ExternalInput")
    W1Bp = nc.dram_tensor("W1Bp", [50, C1], bf16, kind="ExternalInput")
    xbT_rep = nc.dram_tensor("xbT_rep", [50, NPAD], bf16, kind="ExternalInput")
    iotap = nc.dram_tensor("iotap", [P, 1], bf16, kind="ExternalInput")
    xT = nc.dram_tensor("xT", [8, NPAD], bf16, kind="ExternalInput")
    eidx = nc.dram_tensor("eidx", [P, ECOLS], i16, kind="ExternalInput")
    gidx1 = nc.dram_tensor("gidx1", [P, G1COLS], i16, kind="ExternalInput")
    gidx2 = nc.dram_tensor("gidx2", [P, G2COLS], i16, kind="ExternalInput")
    dstlm = nc.dram_tensor("dstlm", [P, NPAIRS], bf16, kind="ExternalInput")
    iotab = nc.dram_tensor("iotab", [P, P], bf16, kind="ExternalInput")
    dstlf = nc.dram_tensor("dstlf", [P, NPAIRS], f32, kind="ExternalInput")
    rdk = nc.dram_tensor("rdk", [1, NLOC], f32, kind="ExternalInput")
    dinv_all = nc.dram_tensor("dinv_all", [P, NTILES], f32, kind="ExternalInput")
    dinvk = nc.dram_tensor("dinvk", [P, TPC], f32, kind="ExternalInput")
    W1hi = nc.dram_tensor("W1hi", [8, C1], bf16, kind="ExternalInput")
    W2 = nc.dram_tensor("W2", [C1, C2], f32, kind="ExternalInput")
    b1f = nc.dram_tensor("b1f", [P, C1], f32, kind="ExternalInput")
    b2f = nc.dram_tensor("b2f", [P, C2], f32, kind="ExternalInput")
    iota = nc.dram_tensor("iota", [P, P], f32, kind="ExternalInput")
    y = nc.dram_tensor("y", [NLOC, C2], f32, kind="ExternalOutput")

    with tile.TileContext(nc) as tc:
        with tc.tile_pool(name="const", bufs=1) as cpool, \
             tc.tile_pool(name="meta", bufs=1) as mpool, \
             tc.tile_pool(name="ge", bufs=6) as gepool, \
             tc.tile_pool(name="xt", bufs=3) as xtpool, \
             tc.tile_pool(name="he1", bufs=7) as he1pool, \
             tc.tile_pool(name="he2", bufs=7) as he2pool, \
             tc.tile_pool(name="sel", bufs=6) as spool, \
             tc.tile_pool(name="epi", bufs=3) as tpool, \
             tc.tile_pool(name="stash", bufs=1) as stpool, \
             tc.tile_pool(name="ptr", bufs=1, space="PSUM") as ptrp, \
             tc.tile_pool(name="pmm", bufs=2, space="PSUM") as pmmp, \
             tc.tile_pool(name="pacc", bufs=5, space="PSUM") as paccp, \
             tc.tile_pool(name="dram", bufs=1, space="DRAM") as dram:

            # ---------- constants ----------
            ident = cpool.tile([P, P], f32, tag="ident")
            make_identity(nc, ident[:])
            identb = cpool.tile([P, P], bf16, tag="identb")
            nc.vector.tensor_copy(out=identb[:], in_=ident[:])
            iota_t = cpool.tile([P, P], f32, tag="iota")
            nc.sync.dma_start(out=iota_t[:], in_=iota[:])
            iotab_t = cpool.tile([P, P], bf16, tag="iotab")
            nc.sync.dma_start(out=iotab_t[:], in_=iotab[:])
            iotap_t = cpool.tile([P, 1], bf16, tag="iotap")
            nc.sync.dma_start(out=iotap_t[:], in_=iotap[:])
            W1Bp_t = cpool.tile([50, C1], bf16, tag="w1bp")
            nc.sync.dma_start(out=W1Bp_t[:], in_=W1Bp[:])
            rdk_t = cpool.tile([1, NLOC], f32, tag="rdk")
            nc.sync.dma_start(out=rdk_t[:], in_=rdk[:])
            W1hi_t = cpool.tile([8, C1], bf16, tag="w1hi")
            nc.sync.dma_start(out=W1hi_t[:], in_=W1hi[:])
            W2t = cpool.tile([C1, C2], f32, tag="w2")
            nc.sync.dma_start(out=W2t[:], in_=W2[:])
            b1t = cpool.tile([P, C1], f32, tag="b1")
            nc.sync.dma_start(out=b1t[:], in_=b1f[:])
            b2t = cpool.tile([P, C2], f32, tag="b2")
            nc.sync.dma_start(out=b2t[:], in_=b2f[:])
            dinvA = cpool.tile([P, NTILES], f32, tag="dinvA")
            nc.sync.dma_start(out=dinvA[:], in_=dinv_all[:])
            dinvK = cpool.tile([P, TPC], f32, tag="dinvK")
            nc.sync.dma_start(out=dinvK[:], in_=dinvk[:])
            eidx_t = mpool.tile([P, ECOLS], i16, tag="eidx")
            nc.sync.dma_start(out=eidx_t[:], in_=eidx[:])
            h2stash = stpool.tile([P, TPC * C2], f32, tag="h2stash")

            table1 = dram.tile([NPAD, C1], bf16, tag="table1")
            ag2 = dram.tile([NLOC, C2], f32, tag="ag2")
            table2 = dram.tile([NPAD, C2], f32, tag="table2")

            gq = [0]

            def next_q():
                q = gq[0] % NQ
                gq[0] += 1
                return q

            # ---------- stage 1 (replicated): build full h1 table ----------
            for e in range(NTILES // GS):
                nidx = GS * P
                ge = gepool.tile([P, GS * P], bf16, tag="ge", name=f"ge_{e}")
                nc.gpsimd.dma_gather(
                    out_ap=ge[:].rearrange("p (n c) -> p n c", c=C1),
                    in_ap=emb_aw[:],
                    idxs_ap=eidx_t[:, e * nidx // 16:(e + 1) * nidx // 16],
                    num_idxs=nidx, num_idxs_reg=nidx, elem_size=C1,
                    single_packet=False, queue_num=next_q())
                xt_c = xtpool.tile([8, GS * P], bf16, tag="xt", name=f"xt_{e}")
                nc.sync.dma_start(out=xt_c[:],
                                  in_=xT[:, e * GS * P:(e + 1) * GS * P])
                xb_c = xtpool.tile([50, GS * P], bf16, tag="xb", name=f"xb_{e}")
                nc.sync.dma_start(out=xb_c[:],
                                  in_=xbT_rep[:, e * GS * P:(e + 1) * GS * P])
                ob = gepool.tile([50, GS * P], bf16, tag="ob", name=f"ob_{e}")
                nc.vector.tensor_tensor(
                    out=ob[:], in0=xb_c[:],
                    in1=iotap_t[0:50, 0:1].to_broadcast([50, GS * P]),
                    op=mybir.AluOpType.is_equal)
                h1st = xtpool.tile([P, GS * C1], bf16, tag="h1st",
                                   name=f"h1st_{e}")
                for j in range(GS):
                    t = e * GS + j
                    php = pmmp if j % 2 == 0 else paccp
                    ph = php.tile([P, C1], f32, space="PSUM",
                                  tag="pmm" if j % 2 == 0 else "pacc",
                                  name=f"ph1_{t}")
                    nc.tensor.matmul(out=ph[:], lhsT=identb[:],
                                     rhs=ge[:, j * C1:(j + 1) * C1],
                                     start=True, stop=False)
                    nc.tensor.matmul(out=ph[:], lhsT=ob[:, j * P:(j + 1) * P],
                                     rhs=W1Bp_t[:], start=False, stop=False)
                    nc.tensor.matmul(out=ph[:], lhsT=xt_c[:, j * P:(j + 1) * P],
                                     rhs=W1hi_t[:], start=False, stop=True)
                    if t % 2 == 0:
                        nc.scalar.activation(
                            out=h1st[:, j * C1:(j + 1) * C1], in_=ph[:],
                            func=mybir.ActivationFunctionType.Copy,
                            scale=dinvA[:, t:t + 1])
                    else:
                        nc.vector.tensor_tensor(
                            out=h1st[:, j * C1:(j + 1) * C1], in0=ph[:],
                            in1=dinvA[:, t:t + 1].to_broadcast([P, C1]),
                            op=mybir.AluOpType.mult)
                nc.sync.dma_start(
                    out=table1[e * GS * P:(e + 1) * GS * P, :].rearrange(
                        "(n p) c -> p n c", p=P),
                    in_=h1st[:].rearrange("p (n c) -> p n c", c=C1))

            # conv metadata loads (overlap stage-1)
            gidx1_t = mpool.tile([P, G1COLS], i16, tag="gidx1")
            nc.sync.dma_start(out=gidx1_t[:], in_=gidx1[:])
            gidx2_t = mpool.tile([P, G2COLS], i16, tag="gidx2")
            nc.sync.dma_start(out=gidx2_t[:], in_=gidx2[:])
            dstl_t = mpool.tile([P, NPAIRS], bf16, tag="dstl")
            nc.sync.dma_start(out=dstl_t[:], in_=dstlm[:])
            dstlf_t = mpool.tile([P, NPAIRS], f32, tag="dstlf")
            nc.sync.dma_start(out=dstlf_t[:], in_=dstlf[:])

            tc.strict_bb_all_engine_barrier()

            # ---------- conv1 ----------
            o1 = 0
            pc = 0
            for k in range(TPC):
                nch = nch_of_k[k]
                nidx = (1 + nch) * P
                he = he1pool.tile([P, NCH1 * C1], bf16, tag="he1",
                                  name=f"he1_{k}")
                nc.gpsimd.dma_gather(
                    out_ap=he[:, 0:(1 + nch) * C1].rearrange(
                        "p (n c) -> p n c", c=C1),
                    in_ap=table1[T1_MID:, :],
                    idxs_ap=gidx1_t[:, o1 // 16:(o1 + nidx) // 16],
                    num_idxs=nidx, num_idxs_reg=nidx, elem_size=C1,
                    single_packet=False, queue_num=next_q())
                o1 += nidx
                pacc = paccp.tile([P, C1], f32, space="PSUM", tag="pacc",
                                  name=f"pacc1_{k}")
                for j in range(nch):
                    S = spool.tile([P, P], bf16, tag="S1", name=f"S1_{k}_{j}")
                    nc.vector.tensor_tensor(
                        out=S[:],
                        in0=dstl_t[:, pc + j:pc + j + 1].to_broadcast([P, P]),
                        in1=iotab_t[:], op=mybir.AluOpType.is_equal)
                    nc.tensor.matmul(
                        out=pacc[:], lhsT=S[:],
                        rhs=he[:, (1 + j) * C1:(2 + j) * C1],
                        start=(j == 0), stop=False)
                pc += nch
                # pacc += self rows; pacc += b1/dinv (so relu(dinv*pacc) is exact)
                nc.tensor.matmul(out=pacc[:], lhsT=identb[:], rhs=he[:, 0:C1],
                                 start=False, stop=False)
                nc.tensor.matmul(out=pacc[:], lhsT=rdk_t[:, k * P:(k + 1) * P],
                                 rhs=b1t[0:1, :], start=False, stop=True)
                t4 = tpool.tile([P, C1], f32, tag="t4", name=f"t4_{k}")
                nc.scalar.activation(out=t4[:], in_=pacc[:],
                                     func=mybir.ActivationFunctionType.Relu,
                                     scale=dinvK[:, k:k + 1])
                # h2 = (t4 @ W2) * dinv
                ptr2 = ptrp.tile([P, P], f32, space="PSUM", tag="ptr",
                                 name=f"ptr2_{k}")
                nc.tensor.transpose(out=ptr2[:], in_=t4[:], identity=ident[:])
                hT = tpool.tile([P, P], f32, tag="hT", name=f"hT_{k}")
                nc.vector.tensor_copy(out=hT[:], in_=ptr2[:])
                ph2 = pmmp.tile([P, C2], f32, space="PSUM", tag="pmm",
                                name=f"ph2_{k}")
                nc.tensor.matmul(out=ph2[:], lhsT=hT[:], rhs=W2t[:],
                                 start=True, stop=True)
                nc.scalar.activation(out=h2stash[:, k * C2:(k + 1) * C2],
                                     in_=ph2[:],
                                     func=mybir.ActivationFunctionType.Copy,
                                     scale=dinvK[:, k:k + 1])
                nc.sync.dma_start(out=ag2[k * P:(k + 1) * P, :],
                                  in_=h2stash[:, k * C2:(k + 1) * C2])

            nc.gpsimd.collective_compute(
                "AllGather", mybir.AluOpType.bypass,
                replica_groups=[list(range(CORES))],
                ins=[ag2.opt()], outs=[table2.opt()])

            # ---------- conv2 ----------
            o2 = 0
            pc = 0
            for k in range(TPC):
                nch = nch_of_k[k]
                nidx = nch * P
                he = he2pool.tile([P, NCH2 * C2], f32, tag="he2",
                                  name=f"he2_{k}")
                nc.gpsimd.dma_gather(
                    out_ap=he[:, 0:nch * C2].rearrange(
                        "p (n c) -> p n c", c=C2),
                    in_ap=table2[T2_MID:, :],
                    idxs_ap=gidx2_t[:, o2 // 16:(o2 + nidx) // 16],
                    num_idxs=nidx, num_idxs_reg=nidx, elem_size=C2,
                    single_packet=False, queue_num=next_q())
                o2 += nidx
                pacc = paccp.tile([P, C2], f32, space="PSUM", tag="pacc",
                                  name=f"pacc2_{k}")
                for j in range(nch):
                    S = spool.tile([P, P], f32, tag="S2", name=f"S2_{k}_{j}")
                    nc.vector.tensor_tensor(
                        out=S[:],
                        in0=dstlf_t[:, pc + j:pc + j + 1].to_broadcast([P, P]),
                        in1=iota_t[:], op=mybir.AluOpType.is_equal)
                    nc.tensor.matmul(
                        out=pacc[:], lhsT=S[:],
                        rhs=he[:, j * C2:(j + 1) * C2],
                        start=(j == 0), stop=False)
                pc += nch
                nc.tensor.matmul(out=pacc[:], lhsT=ident[:],
                                 rhs=h2stash[:, k * C2:(k + 1) * C2],
                                 start=False, stop=False)
                nc.tensor.matmul(out=pacc[:], lhsT=rdk_t[:, k * P:(k + 1) * P],
                                 rhs=b2t[0:1, :], start=False, stop=True)
                t4 = tpool.tile([P, C2], f32, tag="u4", name=f"u4_{k}")
                nc.scalar.activation(out=t4[:], in_=pacc[:],
                                     func=mybir.ActivationFunctionType.Relu,
                                     scale=dinvK[:, k:k + 1])
                nc.sync.dma_start(out=y[k * P:(k + 1) * P, :], in_=t4[:])

    nc.compile()
    return nc


_cache = {}


def kernel(x, edge_index, emb_a, emb_b, W1, b1, W2, b2):
    in_maps, meta = prep(x, edge_index, emb_a, emb_b, W1, b1, W2, b2)
    key = (meta["nch_of_k"], meta["NPAIRS"])
    if key not in _cache:
        _cache[key] = build(meta)
    nc = _cache[key]
    res = run_bass_kernel_spmd(nc, in_maps, core_ids=list(range(CORES)))
    out = np.zeros((N, C2), dtype=np.float32)
    for c in range(CORES):
        yc = res.results[c]["y"]
        nodes = np.concatenate(
            [t * P + np.arange(P) for t in meta["core_tiles"][c]])
        valid = nodes < N
        out[nodes[valid]] = yc[valid]
    return out
